# revision 44
# baseline (speedup 1.0000x reference)
"""Multi-head attention kernel for 8 Trainium2 NeuronCores (bf16 pipeline).

Problem: x[4, 2048, 1024], 16 heads x 64 head-dim MHA, fp32.
Sharding: 8 cores = 4 batches x 2 head-groups (8 heads each); host sums the
two partial outputs per batch and adds bo.

Per-core pipeline (all matmuls bf16, accumulation fp32 in PSUM):
  - x (fp32) loaded, transposed on PE, stored bf16 -> xT [dim, seq].
  - Attention runs per (head-pair, q-half) with a kt-outer loop. Scores for
    the two heads (64-deep contraction each) are issued back-to-back at PE
    row groups 0/64 (tile_position auto-derived from base partitions) so the
    hardware runs them concurrently. exp on ScalarE (psum fp32 -> sbuf bf16,
    two 1024-wide calls per kt, staggered across two 2-bank psum tiles so
    ScalarE never stalls on the PE). attnV accumulates U[65, 512] +=
    Vaug^T P per q-chunk, where Vaug = [V | ones] so row 64 collects the
    softmax denominator; attnV is emitted one kt behind exp so the in-order
    PE queue never head-of-line blocks on ScalarE.
  - All non-attention matmul work is chopped into small "filler" chunks
    emitted inside the attention kt-loops (they borrow one of the staggered
    psum score slots): the V projection fills pass (0,0); K/Q projections of
    the next pair fill later passes; output-projection partials over pairs
    0-2 (accumulated into a bf16 buffer) fill pair 3's passes. This keeps
    ScalarE (the bottleneck engine, ~300us of exp) busy almost continuously.
  - Normalize C = U * (1/denominator) (denominator row broadcast across 64
    partitions via a DRAM roundtrip), stored bf16; the output projection
    tail adds pair 3's contribution to the bf16 partials.
PSUM budget: 4 banks staggered score tiles + 4 banks U accumulators = 8.
"""

import numpy as np

B = 4
SEQ = 2048
DIM = 1024
NH_LOC = 8
HID = 64
HDL = NH_LOC * HID  # 512
N_CORES = 8
N_PAIRS = NH_LOC // 2

_PROG = None


def _build_program(seq=SEQ, reps=1):
    import contextlib

    import concourse.bass as bass
    import concourse.mybir as mybir
    import concourse.tile as tile
    from concourse import bacc
    from concourse.masks import make_identity

    FP32 = mybir.dt.float32
    BF16 = mybir.dt.bfloat16
    FP8 = mybir.dt.float8e4
    Exp = mybir.ActivationFunctionType.Exp
    Alu = mybir.AluOpType
    DR = mybir.MatmulPerfMode.DoubleRow

    seq_t = seq // 128
    n_qc = seq // 512
    n_tp = seq_t // 2
    dim_t = DIM // 128

    nc = bacc.Bacc()
    x_d = nc.declare_dram_parameter("x", [seq, DIM], FP32, isOutput=False)
    wq_d = nc.declare_dram_parameter("wq", [DIM, HDL], BF16, isOutput=False)
    wk_d = nc.declare_dram_parameter("wk", [DIM, HDL], BF16, isOutput=False)
    wv_d = nc.declare_dram_parameter("wv", [DIM, HDL], BF16, isOutput=False)
    wo_d = nc.declare_dram_parameter("wo", [HDL, DIM], BF16, isOutput=False)
    bq_d = nc.declare_dram_parameter("bq", [HDL], FP32, isOutput=False)
    bk_d = nc.declare_dram_parameter("bk", [HDL], FP32, isOutput=False)
    bv_d = nc.declare_dram_parameter("bv", [HDL], FP32, isOutput=False)
    out_d = nc.declare_dram_parameter("out", [seq, DIM], FP32, isOutput=True)
    rrs_d = nc.dram_tensor("rrs", [NH_LOC, n_qc, 512], FP32)

    env = dict(seq=seq, seq_t=seq_t, n_qc=n_qc, n_tp=n_tp, dim_t=dim_t,
               FP32=FP32, BF16=BF16, FP8=FP8, Exp=Exp, Alu=Alu, DR=DR,
               bass=bass, tile=tile, make_identity=make_identity,
               x_d=x_d, wq_d=wq_d, wk_d=wk_d, wv_d=wv_d, wo_d=wo_d,
               bq_d=bq_d, bk_d=bk_d, bv_d=bv_d, out_d=out_d, rrs_d=rrs_d)

    with tile.TileContext(nc, pool_alloc_mode="queue") as tc:
        with tc.tile_pool(name="persist", bufs=1) as persist:
            ident = persist.tile([128, 128], FP32)
            make_identity(nc, ident[:])
            # xT: one big tile, chunk d at cols [d*seq, (d+1)*seq)
            xT = persist.tile([128, dim_t * seq], BF16, tag="xt", name="xt")
            # UT[pair][qc]: C^T slabs, bf16 (head A rows 0:64, B rows 64:128)
            UT = [[persist.tile([128, 512], BF16, tag=f"ut{p}_{q}",
                                name=f"ut{p}_{q}")
                   for q in range(n_qc)] for p in range(N_PAIRS)]
            # Vaug bf16: [p, st, head, 65]; cols 0:64 = V, col 64 = ones
            Vaug = persist.tile([128, seq_t * NH_LOC * 65], BF16, tag="vaug",
                                name="vaug")
            wv_sb = persist.tile([128, dim_t * HDL], BF16, tag="wvsb",
                                 name="wvsb")
            wo_sb = persist.tile([128, N_PAIRS * DIM], BF16, tag="wosb",
                                 name="wosb")
            # bf16 partial sums of the output projection (pairs 0-2)
            pout = persist.tile([128, seq_t * DIM], BF16, tag="pout",
                                name="pout")
            bq_sb = persist.tile([128, N_PAIRS], FP32)
            bk_sb = persist.tile([128, N_PAIRS], FP32)
            bv_bc = persist.tile([128, HDL], FP32)
            nc.sync.dma_start(out=bq_sb[:],
                              in_=bq_d[:].rearrange("(m p) -> p m", p=128))
            nc.sync.dma_start(out=bk_sb[:],
                              in_=bk_d[:].rearrange("(m p) -> p m", p=128))
            bv_ap = bv_d[:]
            nc.sync.dma_start(
                out=bv_bc[:],
                in_=bass.AP(tensor=bv_ap.tensor, offset=bv_ap.offset,
                            ap=[[0, 128], [1, HDL]]))

            env.update(ident=ident, xT=xT, UT=UT, Vaug=Vaug,
                       wv_sb=wv_sb, wo_sb=wo_sb, bq_sb=bq_sb, bk_sb=bk_sb,
                       bv_bc=bv_bc, pout=pout)

            rep_ctx = tc.For_i(0, reps, 1) if reps > 1 else contextlib.nullcontext()
            with rep_ctx:
                _build_body(nc, tc, env)

    nc.compile()
    return nc


def _build_body(nc, tc, env):
    seq, seq_t, n_qc, n_tp, dim_t = (env[k] for k in
                                     ("seq", "seq_t", "n_qc", "n_tp", "dim_t"))
    FP32, BF16, FP8, Exp, Alu, DR = (env[k] for k in
                                     ("FP32", "BF16", "FP8", "Exp", "Alu", "DR"))
    bass = env["bass"]
    ident, xT, UT, Vaug = (env[k] for k in ("ident", "xT", "UT", "Vaug"))
    wv_sb, wo_sb, bq_sb, bk_sb, bv_bc = (env[k] for k in
                                         ("wv_sb", "wo_sb", "bq_sb", "bk_sb",
                                          "bv_bc"))
    pout = env["pout"]
    x_d, wq_d, wk_d, wv_d, wo_d = (env[k] for k in
                                   ("x_d", "wq_d", "wk_d", "wv_d", "wo_d"))
    out_d, rrs_d = env["out_d"], env["rrs_d"]

    Vaug4 = Vaug[:].rearrange("p (st h c) -> p st h c", st=seq_t, c=65)

    with (
        tc.tile_pool(name="xstage", bufs=2) as xstage,
        tc.tile_pool(name="wstage", bufs=2) as wstage,
        tc.tile_pool(name="cstage", bufs=3) as cstage,
        tc.tile_pool(name="k8pool", bufs=2) as k8pool,
        tc.tile_pool(name="q8pool", bufs=2) as q8pool,
        tc.tile_pool(name="e2pool", bufs=3) as e2pool,
        tc.tile_pool(name="rstage", bufs=2) as rstage,
        tc.tile_pool(name="outstage", bufs=2) as outstage,
        tc.tile_pool(name="sps", bufs=2, space="PSUM") as sps,
        tc.tile_pool(name="ups", bufs=1, space="PSUM") as ups,
    ):
        def xT_c(d):
            return xT[:, d*seq:(d+1)*seq]

        def w_colmajor_ap(w_ap, nd, ncols, row_stride):
            # dram W [(d p), c] read as [p, (d, c)]: element (p, d, c) at
            # offset (d*128 + p) * row_stride + c
            return bass.AP(tensor=w_ap.tensor, offset=w_ap.offset,
                           ap=[[row_stride, 128], [128 * row_stride, nd],
                               [1, ncols]])

        nc.vector.memset(Vaug4[:, :, :, HID:HID+1], 1.0)
        nc.sync.dma_start(
            out=wv_sb[:],
            in_=w_colmajor_ap(wv_d[:, :], dim_t, HDL, HDL))

        # ---------- filler chunks (emitted inside attention kt-loops) ------
        def vproj_chunk(st):
            # V projection for seq-tile st (all 8 heads) -> bf16 Vaug
            def run():
                vp = sps.tile([128, 1024], FP32, tag="s2")
                for d in range(dim_t):
                    nc.tensor.matmul(
                        vp[:, 0:512], xT_c(d)[:, st*128:(st+1)*128],
                        wv_sb[:, d*HDL:(d+1)*HDL],
                        start=(d == 0), stop=(d == dim_t - 1))
                nc.vector.tensor_tensor(
                    Vaug4[:, st, :, 0:HID],
                    vp[:, 0:512].rearrange("p (h c) -> p h c", c=HID),
                    bv_bc[:].rearrange("p (h c) -> p h c", c=HID), Alu.add)
            return run

        K8s = [None] * N_PAIRS
        Q8s = [None] * N_PAIRS

        def projK_chunks(p):
            state = {}

            def pre():
                wkp = wstage.tile([128, DIM], BF16, tag="wst")
                nc.sync.dma_start(
                    out=wkp[:],
                    in_=w_colmajor_ap(wk_d[:, p*128:(p+1)*128], dim_t, 128,
                                      HDL))
                K8p = k8pool.tile([128, seq], BF16, tag="k8",
                                  name=f"k8_{p}")
                state["w"] = wkp
                state["kt"] = K8p
                K8s[p] = K8p

            def chunk(sc):
                def run():
                    if sc == 0:
                        pre()
                    wkp, ktile = state["w"], state["kt"]
                    kp = sps.tile([128, 1024], FP32, tag="s2")
                    for d in range(dim_t):
                        nc.tensor.matmul(
                            kp[:, 0:512], wkp[:, d*128:(d+1)*128],
                            xT_c(d)[:, sc*512:(sc+1)*512],
                            start=(d == 0), stop=(d == dim_t - 1))
                    nc.vector.tensor_scalar(ktile[:, sc*512:(sc+1)*512],
                                            kp[:, 0:512],
                                            bk_sb[:, p:p+1], 0.125,
                                            Alu.add, Alu.mult)
                return run
            return [chunk(sc) for sc in range(n_qc)]

        def projQ_chunks(p):
            state = {}

            def pre():
                wqp = wstage.tile([128, DIM], BF16, tag="wst")
                nc.sync.dma_start(
                    out=wqp[:],
                    in_=w_colmajor_ap(wq_d[:, p*128:(p+1)*128], dim_t, 128,
                                      HDL))
                Q8p = q8pool.tile([128, seq], BF16, tag="q8",
                                  name=f"q8_{p}")
                state["w"] = wqp
                state["qt"] = Q8p
                Q8s[p] = Q8p

            def chunk(sc):
                def run():
                    if sc == 0:
                        pre()
                    wqp, qtile = state["w"], state["qt"]
                    qp = sps.tile([128, 1024], FP32, tag="s2")
                    for d in range(dim_t):
                        nc.tensor.matmul(
                            qp[:, 0:512], wqp[:, d*128:(d+1)*128],
                            xT_c(d)[:, sc*512:(sc+1)*512],
                            start=(d == 0), stop=(d == dim_t - 1))
                    nc.vector.tensor_scalar(qtile[:, sc*512:(sc+1)*512],
                                            qp[:, 0:512], bq_sb[:, p:p+1],
                                            None, Alu.add)
                return run
            return [chunk(sc) for sc in range(n_qc)]

        pout5 = pout[:].rearrange("p (st c) -> p st c", c=DIM)

        def outproj_partial_chunk(st, oc):
            # accumulate pairs 0-2 of the output projection into bf16 pout
            def run():
                op = sps.tile([128, 1024], FP32, tag="s2")
                for p in range(N_PAIRS - 1):
                    nc.tensor.matmul(
                        op[:, 0:512],
                        UT[p][st // 4][:, (st % 4)*128:(st % 4)*128+128],
                        wo_sb[:, p*DIM + oc*512:p*DIM + oc*512 + 512],
                        start=(p == 0), stop=(p == N_PAIRS - 2))
                nc.vector.tensor_copy(pout5[:, st, oc*512:(oc+1)*512],
                                      op[:, 0:512])
            return run

        # ---------- Phase A: load x (fp32), transpose via PE -> bf16 xT,
        # with K0/Q0 projection chunks interleaved as soon as their xT
        # q-chunks are complete (cuts the serial startup before head 0).
        k0c = projK_chunks(0)
        q0c = projQ_chunks(0)
        for st in range(seq_t):
            xst = xstage.tile([128, DIM], FP32, tag="xst")
            nc.sync.dma_start(out=xst[:], in_=x_d[st*128:(st+1)*128, :])
            for half in range(2):
                tp = sps.tile([128, 1024], FP32, tag="s2")
                for j in range(4):
                    d = half * 4 + j
                    nc.tensor.transpose(tp[:, j*128:(j+1)*128],
                                        xst[:, d*128:(d+1)*128], ident[:])
                # strided ScalarE copy: 4 transposed blocks -> bf16 xT chunks
                out_ap = xT[:].rearrange("p (d s) -> p d s", d=dim_t)[
                    :, half*4:(half+1)*4, st*128:(st+1)*128]
                nc.scalar.copy(out_ap,
                               tp[:, 0:512].rearrange("p (d c) -> p d c",
                                                      c=128))
            if st % 4 == 3:
                k0c[st // 4]()
                q0c[st // 4]()

        # ---------- Phase C: attention per head ----------
        def attend_pair_half(pair, half, fillers):
            # Both heads of the pair, q-chunks (2*half, 2*half+1). The two
            # heads' 64-contraction score matmuls sit at PE row groups 0/64
            # (tile_position auto-derived) and run concurrently.
            hA, hB = 2 * pair, 2 * pair + 1
            qa, qb = 2 * half, 2 * half + 1
            ktile = K8s[pair]
            qtile = Q8s[pair]
            U = [ups.tile([65, 512], FP32, tag=f"u{i}", name=f"u{i}")
                 for i in range(4)]  # A-qa, B-qa, A-qb, B-qb
            e2refs = [None] * seq_t

            def attnV(kt):
                e2t = e2refs[kt]
                for i, h in enumerate((hA, hB, hA, hB)):
                    nc.tensor.matmul(
                        U[i][:], Vaug4[:, kt, h, 0:HID+1],
                        e2t[:, i*512:(i+1)*512],
                        start=(kt == 0), stop=(kt == seq_t - 1))

            nf = len(fillers)
            done = 0
            for kt in range(seq_t):
                s2a = sps.tile([128, 1024], FP32, tag="s2")
                s2b = sps.tile([128, 1024], FP32, tag="s2")
                kc = slice(kt*128, (kt+1)*128)
                for tgt, q in ((s2a, qa), (s2b, qb)):
                    nc.tensor.matmul(tgt[:, 0:512],
                                     ktile[0:64, kc],
                                     qtile[0:64, q*512:(q+1)*512],
                                     start=True, stop=True)
                    nc.tensor.matmul(tgt[:, 512:1024],
                                     ktile[64:128, kc],
                                     qtile[64:128, q*512:(q+1)*512],
                                     start=True, stop=True)
                e2t = e2pool.tile([128, 2048], BF16, tag="e2")
                e2refs[kt] = e2t
                nc.scalar.activation(e2t[:, 0:1024], s2a[:], Exp)
                nc.scalar.activation(e2t[:, 1024:2048], s2b[:], Exp)
                want = (kt + 1) * nf // seq_t
                while done < want:
                    fillers[done]()
                    done += 1
                if kt >= 1:
                    attnV(kt - 1)
            while done < nf:
                fillers[done]()
                done += 1
            attnV(seq_t - 1)

            # normalize: C = U * (1 / denom-row), denom broadcast via DRAM
            for i, (h, q, p0) in enumerate(((hA, qa, 0), (hB, qa, 64),
                                            (hA, qb, 0), (hB, qb, 64))):
                usb = rstage.tile([65, 512], FP32, tag="usb")
                nc.vector.tensor_copy(usb[:], U[i][:])
                rr = rstage.tile([1, 512], FP32, tag="rr")
                nc.vector.reciprocal(rr[:], usb[64:65, :])
                slot = rrs_d[h, q, :]
                nc.sync.dma_start(out=slot, in_=rr[0:1, :])
                rb = rstage.tile([64, 512], FP32, tag="rb")
                nc.sync.dma_start(
                    out=rb[:],
                    in_=bass.AP(tensor=slot.tensor, offset=slot.offset,
                                ap=[[0, 64], [1, 512]]))
                nc.vector.tensor_tensor(UT[pair][q][p0:p0+64, :],
                                        usb[0:64, :], rb[:], Alu.mult)

        def outproj_tail_chunk(st):
            # pair-3 contribution + bf16 partials -> final tile -> DRAM.
            # Needs UT[3][st//4] (previous pass's normalize) and the (st, *)
            # partial chunks (an earlier pass's fillers).
            def run():
                ot = outstage.tile([128, DIM], FP32, tag="ot")
                for oc in range(2):
                    op = sps.tile([128, 1024], FP32, tag="s2")
                    nc.tensor.matmul(
                        op[:, 0:512],
                        UT[N_PAIRS - 1][st // 4][:, (st % 4)*128:
                                                 (st % 4)*128+128],
                        wo_sb[:, (N_PAIRS-1)*DIM + oc*512:
                              (N_PAIRS-1)*DIM + oc*512 + 512],
                        start=True, stop=True)
                    nc.vector.tensor_tensor(ot[:, oc*512:(oc+1)*512],
                                            pout5[:, st, oc*512:(oc+1)*512],
                                            op[:, 0:512], Alu.add)
                nc.sync.dma_start(out=out_d[st*128:(st+1)*128, :], in_=ot[:])
            return run

        po_chunks = [outproj_partial_chunk(st, oc)
                     for st in range(seq_t) for oc in range(2)]
        tails = [outproj_tail_chunk(st) for st in range(seq_t)]
        fill_sched = {
            (0, 0): [vproj_chunk(st) for st in range(seq_t)],
            (0, 1): projK_chunks(1) + projQ_chunks(1),
            (1, 0): projK_chunks(2),
            (1, 1): projQ_chunks(2),
            (2, 0): projK_chunks(3),
            (2, 1): projQ_chunks(3),
            (3, 0): po_chunks[:seq_t],
            (3, 1): po_chunks[seq_t:] + tails[0:8],
        }
        for pair in range(N_PAIRS):
            for half in range(2):
                attend_pair_half(pair, half, fill_sched[(pair, half)])
            if pair == 2:
                nc.sync.dma_start(
                    out=wo_sb[:],
                    in_=w_colmajor_ap(wo_d[:, :], N_PAIRS, DIM, DIM))

        # ---------- Phase D: remaining output-projection tail ----------
        for st in range(8, seq_t):
            tails[st]()


def _get_program():
    global _PROG
    if _PROG is None:
        _PROG = _build_program()
    return _PROG


def _make_in_maps(inputs):
    import ml_dtypes
    bf = ml_dtypes.bfloat16
    x = np.asarray(inputs["x"], dtype=np.float32)
    in_maps = []
    for c in range(N_CORES):
        b, g = divmod(c, 2)
        sl = slice(g * HDL, (g + 1) * HDL)
        in_maps.append({
            "x": np.ascontiguousarray(x[b]),
            "wq": np.ascontiguousarray(np.asarray(inputs["Wq"], np.float32)[:, sl]).astype(bf),
            "wk": np.ascontiguousarray(np.asarray(inputs["Wk"], np.float32)[:, sl]).astype(bf),
            "wv": np.ascontiguousarray(np.asarray(inputs["Wv"], np.float32)[:, sl]).astype(bf),
            "bq": np.ascontiguousarray(np.asarray(inputs["bq"], np.float32)[sl]),
            "bk": np.ascontiguousarray(np.asarray(inputs["bk"], np.float32)[sl]),
            "bv": np.ascontiguousarray(np.asarray(inputs["bv"], np.float32)[sl]),
            "wo": np.ascontiguousarray(np.asarray(inputs["Wo"], np.float32)[sl, :]).astype(bf),
        })
    return in_maps


def kernel(x, Wq, bq, Wk, bk, Wv, bv, Wo, bo):
    from concourse.bass_utils import run_bass_kernel_spmd

    bo = np.asarray(bo, dtype=np.float32)
    nc = _get_program()
    in_maps = _make_in_maps(dict(x=x, Wq=Wq, bq=bq, Wk=Wk, bk=bk, Wv=Wv,
                                 bv=bv, Wo=Wo, bo=bo))
    res = run_bass_kernel_spmd(nc, in_maps, core_ids=list(range(N_CORES)))
    out = np.empty((B, SEQ, DIM), dtype=np.float32)
    for b in range(B):
        out[b] = res.results[2 * b]["out"] + res.results[2 * b + 1]["out"] + bo
    return out


# revision 45
# speedup vs baseline: 1.1124x; 1.1124x over previous
"""Multi-head attention kernel for 8 Trainium2 NeuronCores (bf16 pipeline).

Problem: x[4, 2048, 1024], 16 heads x 64 head-dim MHA, fp32.
Sharding: 8 cores = 4 batches x 2 head-groups (8 heads each); host sums the
two partial outputs per batch and adds bo.

Per-core pipeline (all matmuls bf16, accumulation fp32 in PSUM):
  - x (fp32) loaded, transposed on PE, stored bf16 -> xT [dim, seq].
  - Attention runs per (head-pair, q-half) with a kt-outer loop. Scores for
    the two heads (64-deep contraction each) are issued back-to-back at PE
    row groups 0/64 (tile_position auto-derived from base partitions) so the
    hardware runs them concurrently. exp on ScalarE (psum fp32 -> sbuf bf16,
    two 1024-wide calls per kt, staggered across two 2-bank psum tiles so
    ScalarE never stalls on the PE). attnV accumulates U[65, 512] +=
    Vaug^T P per q-chunk, where Vaug = [V | ones] so row 64 collects the
    softmax denominator; attnV is emitted one kt behind exp so the in-order
    PE queue never head-of-line blocks on ScalarE.
  - All non-attention matmul work is chopped into small "filler" chunks
    emitted inside the attention kt-loops (they borrow one of the staggered
    psum score slots): the V projection fills pass (0,0); K/Q projections of
    the next pair fill later passes; output-projection partials over pairs
    0-2 (accumulated into a bf16 buffer) fill pair 3's passes. This keeps
    ScalarE (the bottleneck engine, ~300us of exp) busy almost continuously.
  - Normalize C = U * (1/denominator) (denominator row broadcast across 64
    partitions via a DRAM roundtrip), stored bf16; the output projection
    tail adds pair 3's contribution to the bf16 partials.
PSUM budget: 4 banks staggered score tiles + 4 banks U accumulators = 8.
"""

import numpy as np

B = 4
SEQ = 2048
DIM = 1024
NH_LOC = 8
HID = 64
HDL = NH_LOC * HID  # 512
N_CORES = 8
N_PAIRS = NH_LOC // 2

_PROG = None


def _build_program(seq=SEQ, reps=1):
    import contextlib

    import concourse.bass as bass
    import concourse.mybir as mybir
    import concourse.tile as tile
    from concourse import bacc
    from concourse.masks import make_identity

    FP32 = mybir.dt.float32
    BF16 = mybir.dt.bfloat16
    FP8 = mybir.dt.float8e4
    Exp = mybir.ActivationFunctionType.Exp
    Alu = mybir.AluOpType
    DR = mybir.MatmulPerfMode.DoubleRow

    seq_t = seq // 128
    n_qc = seq // 512
    n_tp = seq_t // 2
    dim_t = DIM // 128

    nc = bacc.Bacc()
    x_d = nc.declare_dram_parameter("x", [seq, DIM], FP32, isOutput=False)
    wq_d = nc.declare_dram_parameter("wq", [DIM, HDL], BF16, isOutput=False)
    wk_d = nc.declare_dram_parameter("wk", [DIM, HDL], BF16, isOutput=False)
    wv_d = nc.declare_dram_parameter("wv", [DIM, HDL], BF16, isOutput=False)
    wo_d = nc.declare_dram_parameter("wo", [HDL, DIM], BF16, isOutput=False)
    bq_d = nc.declare_dram_parameter("bq", [HDL], FP32, isOutput=False)
    bk_d = nc.declare_dram_parameter("bk", [HDL], FP32, isOutput=False)
    bv_d = nc.declare_dram_parameter("bv", [HDL], FP32, isOutput=False)
    out_d = nc.declare_dram_parameter("out", [seq, DIM], FP32, isOutput=True)
    rrs_d = nc.dram_tensor("rrs", [NH_LOC, n_qc, 512], FP32)

    env = dict(seq=seq, seq_t=seq_t, n_qc=n_qc, n_tp=n_tp, dim_t=dim_t,
               FP32=FP32, BF16=BF16, FP8=FP8, Exp=Exp, Alu=Alu, DR=DR,
               bass=bass, tile=tile, make_identity=make_identity,
               x_d=x_d, wq_d=wq_d, wk_d=wk_d, wv_d=wv_d, wo_d=wo_d,
               bq_d=bq_d, bk_d=bk_d, bv_d=bv_d, out_d=out_d, rrs_d=rrs_d)

    with tile.TileContext(nc, pool_alloc_mode="queue") as tc:
        with tc.tile_pool(name="persist", bufs=1) as persist:
            ident = persist.tile([128, 128], FP32)
            make_identity(nc, ident[:])
            # xT: one big tile, chunk d at cols [d*seq, (d+1)*seq)
            xT = persist.tile([128, dim_t * seq], BF16, tag="xt", name="xt")
            # UT[pair][qc]: C^T slabs, bf16 (head A rows 0:64, B rows 64:128)
            UT = [[persist.tile([128, 512], BF16, tag=f"ut{p}_{q}",
                                name=f"ut{p}_{q}")
                   for q in range(n_qc)] for p in range(N_PAIRS)]
            # Vaug bf16: [p, st, head, 65]; cols 0:64 = V, col 64 = ones
            Vaug = persist.tile([128, seq_t * NH_LOC * 65], BF16, tag="vaug",
                                name="vaug")
            wv_sb = persist.tile([128, dim_t * HDL], BF16, tag="wvsb",
                                 name="wvsb")
            wo_sb = persist.tile([128, N_PAIRS * DIM], BF16, tag="wosb",
                                 name="wosb")
            # bf16 partial sums of the output projection (pairs 0-2)
            pout = persist.tile([128, seq_t * DIM], BF16, tag="pout",
                                name="pout")
            bq_sb = persist.tile([128, N_PAIRS], FP32)
            bk_sb = persist.tile([128, N_PAIRS], FP32)
            bv_bc = persist.tile([128, HDL], FP32)
            nc.sync.dma_start(out=bq_sb[:],
                              in_=bq_d[:].rearrange("(m p) -> p m", p=128))
            nc.sync.dma_start(out=bk_sb[:],
                              in_=bk_d[:].rearrange("(m p) -> p m", p=128))
            bv_ap = bv_d[:]
            nc.sync.dma_start(
                out=bv_bc[:],
                in_=bass.AP(tensor=bv_ap.tensor, offset=bv_ap.offset,
                            ap=[[0, 128], [1, HDL]]))

            env.update(ident=ident, xT=xT, UT=UT, Vaug=Vaug,
                       wv_sb=wv_sb, wo_sb=wo_sb, bq_sb=bq_sb, bk_sb=bk_sb,
                       bv_bc=bv_bc, pout=pout)

            rep_ctx = tc.For_i(0, reps, 1) if reps > 1 else contextlib.nullcontext()
            with rep_ctx:
                _build_body(nc, tc, env)

    nc.compile()
    return nc


def _build_body(nc, tc, env):
    seq, seq_t, n_qc, n_tp, dim_t = (env[k] for k in
                                     ("seq", "seq_t", "n_qc", "n_tp", "dim_t"))
    FP32, BF16, FP8, Exp, Alu, DR = (env[k] for k in
                                     ("FP32", "BF16", "FP8", "Exp", "Alu", "DR"))
    bass = env["bass"]
    ident, xT, UT, Vaug = (env[k] for k in ("ident", "xT", "UT", "Vaug"))
    wv_sb, wo_sb, bq_sb, bk_sb, bv_bc = (env[k] for k in
                                         ("wv_sb", "wo_sb", "bq_sb", "bk_sb",
                                          "bv_bc"))
    pout = env["pout"]
    x_d, wq_d, wk_d, wv_d, wo_d = (env[k] for k in
                                   ("x_d", "wq_d", "wk_d", "wv_d", "wo_d"))
    out_d, rrs_d = env["out_d"], env["rrs_d"]

    Vaug4 = Vaug[:].rearrange("p (st h c) -> p st h c", st=seq_t, c=65)

    with (
        tc.tile_pool(name="xstage", bufs=2) as xstage,
        tc.tile_pool(name="wstage", bufs=2) as wstage,
        tc.tile_pool(name="cstage", bufs=3) as cstage,
        tc.tile_pool(name="k8pool", bufs=2) as k8pool,
        tc.tile_pool(name="q8pool", bufs=2) as q8pool,
        tc.tile_pool(name="e2pool", bufs=3) as e2pool,
        tc.tile_pool(name="rstage", bufs=2) as rstage,
        tc.tile_pool(name="outstage", bufs=2) as outstage,
        tc.tile_pool(name="sps", bufs=2, space="PSUM") as sps,
        tc.tile_pool(name="ups", bufs=1, space="PSUM") as ups,
    ):
        def xT_c(d):
            return xT[:, d*seq:(d+1)*seq]

        def w_colmajor_ap(w_ap, nd, ncols, row_stride):
            # dram W [(d p), c] read as [p, (d, c)]: element (p, d, c) at
            # offset (d*128 + p) * row_stride + c
            return bass.AP(tensor=w_ap.tensor, offset=w_ap.offset,
                           ap=[[row_stride, 128], [128 * row_stride, nd],
                               [1, ncols]])

        nc.vector.memset(Vaug4[:, :, :, HID:HID+1], 1.0)
        nc.sync.dma_start(
            out=wv_sb[:],
            in_=w_colmajor_ap(wv_d[:, :], dim_t, HDL, HDL))

        # ---------- filler chunks (emitted inside attention kt-loops) ------
        def vproj_chunk(st):
            # V projection for seq-tile st (all 8 heads) -> bf16 Vaug
            def run():
                vp = sps.tile([128, 1024], FP32, tag="s2")
                for d in range(dim_t):
                    nc.tensor.matmul(
                        vp[:, 0:512], xT_c(d)[:, st*128:(st+1)*128],
                        wv_sb[:, d*HDL:(d+1)*HDL],
                        start=(d == 0), stop=(d == dim_t - 1))
                nc.vector.tensor_tensor(
                    Vaug4[:, st, :, 0:HID],
                    vp[:, 0:512].rearrange("p (h c) -> p h c", c=HID),
                    bv_bc[:].rearrange("p (h c) -> p h c", c=HID), Alu.add)
            return run

        K8s = [None] * N_PAIRS
        Q8s = [None] * N_PAIRS

        def projK_chunks(p):
            state = {}

            def pre():
                wkp = wstage.tile([128, DIM], BF16, tag="wst")
                nc.sync.dma_start(
                    out=wkp[:],
                    in_=w_colmajor_ap(wk_d[:, p*128:(p+1)*128], dim_t, 128,
                                      HDL))
                K8p = k8pool.tile([128, seq], BF16, tag="k8",
                                  name=f"k8_{p}")
                state["w"] = wkp
                state["kt"] = K8p
                K8s[p] = K8p

            def chunk(sc):
                def run():
                    if sc == 0:
                        pre()
                    wkp, ktile = state["w"], state["kt"]
                    kp = sps.tile([128, 1024], FP32, tag="s2")
                    for d in range(dim_t):
                        nc.tensor.matmul(
                            kp[:, 0:512], wkp[:, d*128:(d+1)*128],
                            xT_c(d)[:, sc*512:(sc+1)*512],
                            start=(d == 0), stop=(d == dim_t - 1))
                    nc.vector.tensor_scalar(ktile[:, sc*512:(sc+1)*512],
                                            kp[:, 0:512],
                                            bk_sb[:, p:p+1], 0.125,
                                            Alu.add, Alu.mult)
                return run
            return [chunk(sc) for sc in range(n_qc)]

        def projQ_chunks(p):
            state = {}

            def pre():
                wqp = wstage.tile([128, DIM], BF16, tag="wst")
                nc.sync.dma_start(
                    out=wqp[:],
                    in_=w_colmajor_ap(wq_d[:, p*128:(p+1)*128], dim_t, 128,
                                      HDL))
                Q8p = q8pool.tile([128, seq], BF16, tag="q8",
                                  name=f"q8_{p}")
                state["w"] = wqp
                state["qt"] = Q8p
                Q8s[p] = Q8p

            def chunk(sc):
                def run():
                    if sc == 0:
                        pre()
                    wqp, qtile = state["w"], state["qt"]
                    qp = sps.tile([128, 1024], FP32, tag="s2")
                    for d in range(dim_t):
                        nc.tensor.matmul(
                            qp[:, 0:512], wqp[:, d*128:(d+1)*128],
                            xT_c(d)[:, sc*512:(sc+1)*512],
                            start=(d == 0), stop=(d == dim_t - 1))
                    nc.vector.tensor_scalar(qtile[:, sc*512:(sc+1)*512],
                                            qp[:, 0:512], bq_sb[:, p:p+1],
                                            None, Alu.add)
                return run
            return [chunk(sc) for sc in range(n_qc)]

        pout5 = pout[:].rearrange("p (st c) -> p st c", c=DIM)

        def outproj_partial_chunk(st, oc):
            # accumulate pairs 0-2 of the output projection into bf16 pout
            def run():
                op = sps.tile([128, 1024], FP32, tag="s2")
                for p in range(N_PAIRS - 1):
                    nc.tensor.matmul(
                        op[:, 0:512],
                        UT[p][st // 4][:, (st % 4)*128:(st % 4)*128+128],
                        wo_sb[:, p*DIM + oc*512:p*DIM + oc*512 + 512],
                        start=(p == 0), stop=(p == N_PAIRS - 2))
                nc.vector.tensor_copy(pout5[:, st, oc*512:(oc+1)*512],
                                      op[:, 0:512])
            return run

        # ---------- Phase A: load x (fp32), transpose via PE -> bf16 xT,
        # with K0/Q0 projection chunks interleaved as soon as their xT
        # q-chunks are complete (cuts the serial startup before head 0).
        k0c = projK_chunks(0)
        q0c = projQ_chunks(0)
        for st in range(seq_t):
            xst = xstage.tile([128, DIM], FP32, tag="xst")
            nc.sync.dma_start(out=xst[:], in_=x_d[st*128:(st+1)*128, :])
            for half in range(2):
                tp = sps.tile([128, 1024], FP32, tag="s2")
                for j in range(4):
                    d = half * 4 + j
                    nc.tensor.transpose(tp[:, j*128:(j+1)*128],
                                        xst[:, d*128:(d+1)*128], ident[:])
                # strided ScalarE copy: 4 transposed blocks -> bf16 xT chunks
                out_ap = xT[:].rearrange("p (d s) -> p d s", d=dim_t)[
                    :, half*4:(half+1)*4, st*128:(st+1)*128]
                nc.scalar.copy(out_ap,
                               tp[:, 0:512].rearrange("p (d c) -> p d c",
                                                      c=128))
            if st % 4 == 3:
                k0c[st // 4]()
                q0c[st // 4]()

        # ---------- Phase C: attention per head ----------
        def attend_pair_half(pair, half, fillers):
            # Both heads of the pair, q-chunks (2*half, 2*half+1). The two
            # heads' 64-contraction score matmuls sit at PE row groups 0/64
            # (tile_position auto-derived) and run concurrently.
            hA, hB = 2 * pair, 2 * pair + 1
            qa, qb = 2 * half, 2 * half + 1
            ktile = K8s[pair]
            qtile = Q8s[pair]
            U = [ups.tile([65, 512], FP32, tag=f"u{i}", name=f"u{i}")
                 for i in range(4)]  # A-qa, B-qa, A-qb, B-qb
            e2refs = [None] * seq_t

            def attnV(kt):
                e2t = e2refs[kt]
                for i, h in enumerate((hA, hB, hA, hB)):
                    nc.tensor.matmul(
                        U[i][:], Vaug4[:, kt, h, 0:HID+1],
                        e2t[:, i*512:(i+1)*512],
                        start=(kt == 0), stop=(kt == seq_t - 1))

            nf = len(fillers)
            done = 0
            for kt in range(seq_t):
                s2a = sps.tile([128, 1024], FP32, tag="s2")
                s2b = sps.tile([128, 1024], FP32, tag="s2")
                kc = slice(kt*128, (kt+1)*128)
                for tgt, q in ((s2a, qa), (s2b, qb)):
                    nc.tensor.matmul(tgt[:, 0:512],
                                     ktile[0:64, kc],
                                     qtile[0:64, q*512:(q+1)*512],
                                     start=True, stop=True)
                    nc.tensor.matmul(tgt[:, 512:1024],
                                     ktile[64:128, kc],
                                     qtile[64:128, q*512:(q+1)*512],
                                     start=True, stop=True)
                e2t = e2pool.tile([128, 2048], BF16, tag="e2")
                e2refs[kt] = e2t
                nc.scalar.activation(e2t[:, 0:1024], s2a[:], Exp)
                nc.scalar.activation(e2t[:, 1024:2048], s2b[:], Exp)
                want = (kt + 1) * nf // seq_t
                while done < want:
                    fillers[done]()
                    done += 1
                if kt >= 1:
                    attnV(kt - 1)
            while done < nf:
                fillers[done]()
                done += 1
            attnV(seq_t - 1)

            # normalize: C = U * (1 / denom-row), denom broadcast via DRAM
            for i, (h, q, p0) in enumerate(((hA, qa, 0), (hB, qa, 64),
                                            (hA, qb, 0), (hB, qb, 64))):
                usb = rstage.tile([65, 512], FP32, tag="usb")
                nc.vector.tensor_copy(usb[:], U[i][:])
                # broadcast the raw denominator row across 64 partitions via
                # a DRAM roundtrip, then approximate 1/denom full-width
                # (reciprocal_approx_fast miscomputes on 1-partition APs).
                slot = rrs_d[h, q, :]
                nc.sync.dma_start(out=slot, in_=usb[64:65, :])
                rb = rstage.tile([64, 512], FP32, tag="rb")
                nc.sync.dma_start(
                    out=rb[:],
                    in_=bass.AP(tensor=slot.tensor, offset=slot.offset,
                                ap=[[0, 64], [1, 512]]))
                rrb = rstage.tile([64, 512], FP32, tag="rrb")
                nc.vector.reciprocal_approx_fast(rrb[:], rb[:])
                nc.vector.tensor_tensor(UT[pair][q][p0:p0+64, :],
                                        usb[0:64, :], rrb[:], Alu.mult)

        def outproj_tail_chunk(st):
            # pair-3 contribution + bf16 partials -> final tile -> DRAM.
            # Needs UT[3][st//4] (previous pass's normalize) and the (st, *)
            # partial chunks (an earlier pass's fillers).
            def run():
                ot = outstage.tile([128, DIM], FP32, tag="ot")
                for oc in range(2):
                    op = sps.tile([128, 1024], FP32, tag="s2")
                    nc.tensor.matmul(
                        op[:, 0:512],
                        UT[N_PAIRS - 1][st // 4][:, (st % 4)*128:
                                                 (st % 4)*128+128],
                        wo_sb[:, (N_PAIRS-1)*DIM + oc*512:
                              (N_PAIRS-1)*DIM + oc*512 + 512],
                        start=True, stop=True)
                    nc.vector.tensor_tensor(ot[:, oc*512:(oc+1)*512],
                                            pout5[:, st, oc*512:(oc+1)*512],
                                            op[:, 0:512], Alu.add)
                nc.sync.dma_start(out=out_d[st*128:(st+1)*128, :], in_=ot[:])
            return run

        po_chunks = [outproj_partial_chunk(st, oc)
                     for st in range(seq_t) for oc in range(2)]
        tails = [outproj_tail_chunk(st) for st in range(seq_t)]
        fill_sched = {
            (0, 0): [vproj_chunk(st) for st in range(seq_t)],
            (0, 1): projK_chunks(1) + projQ_chunks(1),
            (1, 0): projK_chunks(2),
            (1, 1): projQ_chunks(2),
            (2, 0): projK_chunks(3),
            (2, 1): projQ_chunks(3),
            (3, 0): po_chunks[:seq_t],
            (3, 1): po_chunks[seq_t:] + tails[0:8],
        }
        for pair in range(N_PAIRS):
            for half in range(2):
                attend_pair_half(pair, half, fill_sched[(pair, half)])
            if pair == 2:
                nc.sync.dma_start(
                    out=wo_sb[:],
                    in_=w_colmajor_ap(wo_d[:, :], N_PAIRS, DIM, DIM))

        # ---------- Phase D: remaining output-projection tail ----------
        for st in range(8, seq_t):
            tails[st]()


def _get_program():
    global _PROG
    if _PROG is None:
        _PROG = _build_program()
    return _PROG


def _make_in_maps(inputs):
    import ml_dtypes
    bf = ml_dtypes.bfloat16
    x = np.asarray(inputs["x"], dtype=np.float32)
    in_maps = []
    for c in range(N_CORES):
        b, g = divmod(c, 2)
        sl = slice(g * HDL, (g + 1) * HDL)
        in_maps.append({
            "x": np.ascontiguousarray(x[b]),
            "wq": np.ascontiguousarray(np.asarray(inputs["Wq"], np.float32)[:, sl]).astype(bf),
            "wk": np.ascontiguousarray(np.asarray(inputs["Wk"], np.float32)[:, sl]).astype(bf),
            "wv": np.ascontiguousarray(np.asarray(inputs["Wv"], np.float32)[:, sl]).astype(bf),
            "bq": np.ascontiguousarray(np.asarray(inputs["bq"], np.float32)[sl]),
            "bk": np.ascontiguousarray(np.asarray(inputs["bk"], np.float32)[sl]),
            "bv": np.ascontiguousarray(np.asarray(inputs["bv"], np.float32)[sl]),
            "wo": np.ascontiguousarray(np.asarray(inputs["Wo"], np.float32)[sl, :]).astype(bf),
        })
    return in_maps


def kernel(x, Wq, bq, Wk, bk, Wv, bv, Wo, bo):
    from concourse.bass_utils import run_bass_kernel_spmd

    bo = np.asarray(bo, dtype=np.float32)
    nc = _get_program()
    in_maps = _make_in_maps(dict(x=x, Wq=Wq, bq=bq, Wk=Wk, bk=bk, Wv=Wv,
                                 bv=bv, Wo=Wo, bo=bo))
    res = run_bass_kernel_spmd(nc, in_maps, core_ids=list(range(N_CORES)))
    out = np.empty((B, SEQ, DIM), dtype=np.float32)
    for b in range(B):
        out[b] = res.results[2 * b]["out"] + res.results[2 * b + 1]["out"] + bo
    return out
